# revision 20
# baseline (speedup 1.0000x reference)
"""Trainium2 Bass kernel for nn_DigitConvolutionalModel (3x3 conv + 3-layer MLP).

Math: out = relu(relu(conv3x3(x) @ W1 + b1) @ W2 + b2) @ W3 + b3.

The 3x3 valid conv is linear, so on host we fold it into the first FC:
  h1 = relu(x @ W1eff + b1)  with  W1eff = A @ W1 : [784, 256].
K = 784 is used EXACTLY (6 full 128-row k-tiles + one 16-row tail tile) --
no zero-pad DMA traffic.

Sharding: pure data parallel over the batch across 8 cores (2048 rows each).
Feature-major 3-layer MLP (activations transposed; zero on-device transposes):
  h1T = relu(W1eff.T @ xT + b1)   [256, 2048]
  h2T = relu(W2.T   @ h1T + b2)   [256, 2048]
  oT  =      W3.T   @ h2T + b3    [10, 2048]
Matmuls in fp16 (full-rate PE) with fp32 PSUM accumulation.

DMA discipline (measured): the two HWDGE rings share the same 16 DMA
engines (round-robin per transfer, ~336 GB/s aggregate), each transfer on a
queue costs ~0.5-0.7us of descriptor-generation dead time, and a queue's
first transfer starts ~2.3us after its dma_start issues.  So chunk-0 is
split into just 2-3 medium transfers per ring ordered exactly by first
consumption, the 16-row k6 tail of ALL chunks plus W1's tail rides ONE
small [16, 2304] transfer, and w2|w3 are packed into one tensor.  Nothing
non-critical enters a queue ahead of chunk-0 data.  L1 runs k-outer/m-inner
so both m-tiles consume each arriving x k-tile immediately; warm-up matmuls
on a zeroed tile release the PE HAM clock-gate during the fill.
"""

import numpy as np

import concourse.bacc as bacc
import concourse.bass as bass
import concourse.mybir as mybir
import concourse.tile as tile
from concourse.bass_utils import run_bass_kernel_spmd

N_CORES = 8
B = 16384
B_LOC = B // N_CORES  # 2048 batch rows per core
NCH = 512  # batch chunk per matmul (fp32 PSUM bank = 512 floats)
NCHUNKS = B_LOC // NCH
KIN = 784  # folded input features (28*28)
NK = 6  # full 128-row k-tiles; tile 6 is the 16-row tail
KTAIL = KIN - NK * 128  # 16
H = 256
NOUT = 10
NWARM = 30  # small PE warm-up matmuls during the DMA fill

F32 = mybir.dt.float32
F16 = mybir.dt.float16
AF = mybir.ActivationFunctionType
ALU = mybir.AluOpType


def build_nc() -> bass.Bass:
    nc = bacc.Bacc(
        "TRN2", target_bir_lowering=False, debug=False, num_devices=N_CORES
    )
    # Host-packed inputs (exact SBUF destination layouts):
    #   xall[ci][p][k*NCH+n] = x_shard[ci*NCH+n, k*128+p]      k = 0..5
    #   xt6[p][c]: cols 0-255 = W1eff[768+p, c]; cols 256+ci*NCH+n =
    #              x_shard[ci*NCH+n, 768+p]                    p < 16
    #   w1[p][k*256+c]  = W1eff[k*128+p, c]   (k < 6; m0|m1 = c 0..255)
    #   wA: cols 0-511 = k-major W2, cols 512-531 = k-major W3
    #   bias cols: 0-1 = b1(m), 2-3 = b2(m), 4 = b3 (first 10 rows)
    xall = nc.dram_tensor("xall", [NCHUNKS, 128, NK * NCH], F16, kind="ExternalInput")
    xt6 = nc.dram_tensor(
        "xt6", [KTAIL, 256 + NCHUNKS * NCH], F16, kind="ExternalInput"
    )
    w1 = nc.dram_tensor("w1", [128, NK * 256], F16, kind="ExternalInput")
    wA = nc.dram_tensor("wA", [128, 2 * H + 2 * NOUT], F16, kind="ExternalInput")
    bias = nc.dram_tensor("bias", [128, 5], F32, kind="ExternalInput")
    # chunk-major output: each chunk's [10, NCH] block is contiguous in
    # DRAM so a store is one coalesced region (cheap descriptors)
    outC = nc.dram_tensor("outC", [NCHUNKS, NOUT, NCH], F32, kind="ExternalOutput")

    with tile.TileContext(nc) as tc:
        with (
            tc.tile_pool(name="wgt", bufs=1) as wp,
            tc.tile_pool(name="c0", bufs=1) as cp,
            tc.tile_pool(name="xin", bufs=2) as xp,
            tc.tile_pool(name="act", bufs=3) as hp,
            tc.tile_pool(name="osb", bufs=2) as op,
            tc.tile_pool(name="ps1", bufs=2, space="PSUM") as pp1,
            tc.tile_pool(name="ps2", bufs=2, space="PSUM") as pp2,
        ):
            # PE warm-up: small matmuls on a zeroed scratch tile, no DMA
            # deps: a tiny tile memsets fast (earlier sem -> earlier HAM
            # release) and N=128 gives a fine-grained bridge to data arrival.
            warm = wp.tile([128, 128], F16, name="warm")
            nc.vector.memset(warm[:], 0.0)
            psw = pp1.tile([128, NCH], F32, name="psw", tag="ps1_0")
            for _ in range(NWARM):
                nc.tensor.matmul(
                    psw[:, 0:128], warm[:], warm[:], start=True, stop=True
                )

            # ---- chunk-0-critical loads: 2-3 medium transfers per ring,
            # in exact consumption order ----
            w1A = wp.tile([128, 4 * 256], F16, name="w1A")  # k0..k3 weights
            w1B = wp.tile([128, 2 * 256], F16, name="w1B")  # k4..k5 weights
            t01 = cp.tile([128, 2 * NCH], F16, name="x0_01")
            t23 = cp.tile([128, 2 * NCH], F16, name="x0_23")
            t45 = cp.tile([128, 2 * NCH], F16, name="x0_45")
            t6 = wp.tile([KTAIL, 256 + NCHUNKS * NCH], F16, name="t6")

            # Q1 (sync ring): w1_k0123, x0_k23, w1_k45, then xa prefetches
            nc.sync.dma_start(out=w1A[:], in_=w1[:, 0 : 4 * 256])
            nc.sync.dma_start(out=t23[:], in_=xall[0, :, 2 * NCH : 4 * NCH])
            nc.sync.dma_start(out=w1B[:], in_=w1[:, 4 * 256 : 6 * 256])
            # Q10 (scalar ring): x0_k01, x0_k45, tail, bias, then xb
            nc.scalar.dma_start(out=t01[:], in_=xall[0, :, 0 : 2 * NCH])
            nc.scalar.dma_start(out=t45[:], in_=xall[0, :, 4 * NCH : 6 * NCH])
            was = wp.tile([128, 2 * H + 2 * NOUT], F16, name="was")
            nc.scalar.dma_start(out=t6[:], in_=xt6[:, :])
            bs = wp.tile([128, 5], F32, name="bs")
            nc.scalar.dma_start(out=bs[:], in_=bias[:, :])

            # Per-engine bias staging (consumer then depends on its own
            # engine in program order instead of an extra DMA semaphore).
            ba = wp.tile([128, 5], F32, name="ba")  # ACT's copy
            nc.scalar.activation(ba[:], bs[:], AF.Copy)
            bv = wp.tile([128, 5], F32, name="bv")  # DVE's copy
            nc.vector.tensor_copy(bv[:], bs[:])
            b1a = [ba[:, 0:1], ba[:, 1:2]]
            b2a = [ba[:, 2:3], ba[:, 3:4]]
            b1v = [bv[:, 0:1], bv[:, 1:2]]
            b2v = [bv[:, 2:3], bv[:, 3:4]]
            b3v = bv[0:NOUT, 4:5]

            def x_piece(ci, k, xa_t, xb_t):
                """rhs view for k-tile k of chunk ci."""
                if k == NK:
                    return t6[:, 256 + ci * NCH : 256 + (ci + 1) * NCH]
                if ci == 0:
                    blk, off = (t01, 0) if k < 2 else (t23, 2) if k < 4 else (t45, 4)
                    return blk[:, (k - off) * NCH : (k - off + 1) * NCH]
                if k < 3:
                    return xa_t[:, k * NCH : (k + 1) * NCH]
                return xb_t[:, (k - 3) * NCH : (k - 2) * NCH]

            def w1_piece(k, m):
                if k == NK:
                    return t6[0:KTAIL, m * 128 : (m + 1) * 128]
                if k < 4:
                    return w1A[:, k * 256 + m * 128 : k * 256 + (m + 1) * 128]
                return w1B[:, (k - 4) * 256 + m * 128 : (k - 4) * 256 + (m + 1) * 128]

            # ---- batch-chunk pipeline (x prefetched one chunk ahead) ----
            xa_t = xb_t = None
            for ci in range(NCHUNKS):
                n0 = ci * NCH
                cur = (xa_t, xb_t)

                # layer 1, k-outer/m-inner: each x k-tile is consumed by
                # both m matmuls the moment it lands
                ps1 = [
                    pp1.tile([128, NCH], F32, name="ps1", tag=f"ps1_{m}")
                    for m in range(2)
                ]
                # accumulation order is free: for prefetched chunks, slot
                # the (already-resident) k6 tail before k3 so it covers the
                # xb-arrival wait
                ks = list(range(NK + 1)) if ci == 0 else [0, 1, 2, NK, 3, 4, 5]
                for j, k in enumerate(ks):
                    xv = x_piece(ci, k, *cur)
                    for m in range(2):
                        nc.tensor.matmul(
                            ps1[m][:],
                            w1_piece(k, m),
                            xv,
                            start=(j == 0),
                            stop=(j == NK),
                        )
                if ci + 1 < NCHUNKS:
                    xa_t = xp.tile([128, 3 * NCH], F16, name="xa", tag="xa")
                    nc.sync.dma_start(out=xa_t[:], in_=xall[ci + 1, :, 0 : 3 * NCH])
                    xb_t = xp.tile([128, 3 * NCH], F16, name="xb", tag="xb")
                    nc.scalar.dma_start(
                        out=xb_t[:], in_=xall[ci + 1, :, 3 * NCH : 6 * NCH]
                    )
                if ci == 0:
                    nc.sync.dma_start(out=was[:], in_=wA[:, :])

                h1 = []
                for m in range(2):
                    h = hp.tile([128, NCH], F16, name="h1", tag=f"h1_{m}")
                    if m == 0:
                        nc.scalar.activation(h[:], ps1[m][:], AF.Relu, bias=b1a[m])
                    else:
                        nc.vector.tensor_scalar(
                            h[:], ps1[m][:], b1v[m], 0.0, ALU.add, ALU.max
                        )
                    h1.append(h)

                # layer 2: h2T = relu(W2.T @ h1T + b2)
                h2 = []
                for m in range(2):
                    ps = pp2.tile([128, NCH], F32, name="ps2", tag=f"ps2_{m}")
                    for k in range(2):
                        nc.tensor.matmul(
                            ps[:],
                            was[:, k * H + m * 128 : k * H + (m + 1) * 128],
                            h1[k][:],
                            start=(k == 0),
                            stop=(k == 1),
                        )
                    h = hp.tile([128, NCH], F16, name="h2", tag=f"h2_{m}")
                    if m == 0:
                        nc.scalar.activation(h[:], ps[:], AF.Relu, bias=b2a[m])
                    else:
                        nc.vector.tensor_scalar(
                            h[:], ps[:], b2v[m], 0.0, ALU.add, ALU.max
                        )
                    h2.append(h)

                # layer 3: oT = W3.T @ h2T + b3 (shares ps2_1's bank slots)
                ps3f = pp2.tile([128, NCH], F32, name="ps3", tag="ps2_1")
                ps = ps3f[0:NOUT, :]
                for k in range(2):
                    nc.tensor.matmul(
                        ps,
                        was[:, 2 * H + k * NOUT : 2 * H + (k + 1) * NOUT],
                        h2[k][:],
                        start=(k == 0),
                        stop=(k == 1),
                    )
                ob = op.tile([NOUT, NCH], F32, name="ob", tag="ob")
                nc.vector.tensor_scalar(ob[:], ps, b3v, None, ALU.add)
                # stores ride SWDGE so they never block HWDGE loads; the
                # last one rides the (by-then idle) sync ring as a single
                # contiguous packet (fast descriptor generation)
                if ci < NCHUNKS - 1:
                    nc.gpsimd.dma_start(out=outC[ci], in_=ob[:])
                else:
                    nc.sync.dma_start(out=outC[ci], in_=ob[:], single_packet=True)

    nc.compile()
    return nc


def _fold_conv_into_w1(conv_w: np.ndarray, W1: np.ndarray) -> np.ndarray:
    """W1eff[784, 256] such that x @ W1eff == conv_flat(x, conv_w) @ W1."""
    W1v = W1.astype(np.float64).reshape(26, 26, W1.shape[1])
    cw = conv_w.astype(np.float64)
    acc = np.zeros((28, 28, W1.shape[1]), np.float64)
    for di in range(3):
        for dj in range(3):
            acc[di : di + 26, dj : dj + 26, :] += cw[di, dj] * W1v
    return acc.reshape(KIN, W1.shape[1]).astype(np.float32)


def _pack_kmajor(w: np.ndarray, kpad: int) -> np.ndarray:
    """[K, C] -> [128, (K/128)*C] with row-block k at column block k."""
    k, c = w.shape
    wp = np.zeros((kpad, c), w.dtype)
    wp[:k] = w
    return np.ascontiguousarray(
        wp.reshape(kpad // 128, 128, c).transpose(1, 0, 2).reshape(128, -1)
    )


def _run(inputs: dict, trace: bool = False, tmpdir: str | None = None):
    x = np.asarray(inputs["x"], dtype=np.float32)
    w1e = _fold_conv_into_w1(
        np.asarray(inputs["conv_w"]), np.asarray(inputs["W1"])
    ).astype(np.float16)
    w1P = _pack_kmajor(w1e[: NK * 128], NK * 128)  # [128, 6*256]
    w2P = _pack_kmajor(np.asarray(inputs["W2"], np.float16), H)
    w3P = _pack_kmajor(np.asarray(inputs["W3"], np.float16), H)
    wAP = np.ascontiguousarray(np.concatenate([w2P, w3P], axis=1))
    bias = np.zeros((128, 5), np.float32)
    bias[:, 0:2] = np.asarray(inputs["b1"], np.float32).reshape(2, 128).T
    bias[:, 2:4] = np.asarray(inputs["b2"], np.float32).reshape(2, 128).T
    bias[:NOUT, 4] = np.asarray(inputs["b3"], np.float32)

    nc = build_nc()
    in_maps = []
    for c in range(N_CORES):
        xs = x[c * B_LOC : (c + 1) * B_LOC].astype(np.float16)  # [2048, 784]
        # [ci, n, k, p] -> [ci, p, k, n] for the 6 full k-tiles
        arr = xs[:, : NK * 128].reshape(NCHUNKS, NCH, NK, 128)
        xac = np.ascontiguousarray(
            arr.transpose(0, 3, 1, 2).transpose(0, 1, 3, 2).reshape(NCHUNKS, 128, -1)
        )
        xt6c = np.zeros((KTAIL, 256 + NCHUNKS * NCH), np.float16)
        xt6c[:, :256] = w1e[NK * 128 :]
        xt6c[:, 256:] = xs[:, NK * 128 :].T  # [16, 2048]
        in_maps.append(
            {"xall": xac, "xt6": xt6c, "w1": w1P, "wA": wAP, "bias": bias}
        )

    try:
        res = run_bass_kernel_spmd(
            nc, in_maps, list(range(N_CORES)), trace=trace, tmpdir=tmpdir
        )
    except Exception:
        # A prior session can leave a NeuronCore wedged
        # (NRT_EXEC_UNIT_UNRECOVERABLE); a retry with core reset recovers.
        import os

        os.environ.setdefault("NEURON_RT_RESET_CORES", "1")
        res = run_bass_kernel_spmd(
            nc, in_maps, list(range(N_CORES)), trace=trace, tmpdir=tmpdir
        )
    out = np.concatenate(
        [
            r["outC"].transpose(1, 0, 2).reshape(NOUT, B_LOC).T
            for r in res.results
        ],
        axis=0,
    )
    return np.ascontiguousarray(out.astype(np.float32)), res


def kernel(**inputs) -> np.ndarray:
    out, _ = _run(inputs, trace=False)
    return out


# revision 21
# speedup vs baseline: 1.0531x; 1.0531x over previous
"""Trainium2 Bass kernel for nn_DigitConvolutionalModel (3x3 conv + 3-layer MLP).

Math: out = relu(relu(conv3x3(x) @ W1 + b1) @ W2 + b2) @ W3 + b3.

The 3x3 valid conv is linear, so on host we fold it into the first FC:
  h1 = relu(x @ W1eff + b1)  with  W1eff = A @ W1 : [784, 256].
K = 784 is used EXACTLY (6 full 128-row k-tiles + one 16-row tail tile) --
no zero-pad DMA traffic.

Sharding: pure data parallel over the batch across 8 cores (2048 rows each).
Feature-major 3-layer MLP (activations transposed; zero on-device transposes):
  h1T = relu(W1eff.T @ xT + b1)   [256, 2048]
  h2T = relu(W2.T   @ h1T + b2)   [256, 2048]
  oT  =      W3.T   @ h2T + b3    [10, 2048]
Matmuls in fp16 (full-rate PE) with fp32 PSUM accumulation.

DMA discipline (measured): the two HWDGE rings share the same 16 DMA
engines (round-robin per transfer, ~336 GB/s aggregate), each transfer on a
queue costs ~0.5-0.7us of descriptor-generation dead time, and a queue's
first transfer starts ~2.3us after its dma_start issues.  So chunk-0 is
split into just 2-3 medium transfers per ring ordered exactly by first
consumption, the 16-row k6 tail of ALL chunks plus W1's tail rides ONE
small [16, 2304] transfer, and w2|w3 are packed into one tensor.  Nothing
non-critical enters a queue ahead of chunk-0 data.  L1 runs k-outer/m-inner
so both m-tiles consume each arriving x k-tile immediately; warm-up matmuls
on a zeroed tile release the PE HAM clock-gate during the fill.
"""

import numpy as np

import concourse.bacc as bacc
import concourse.bass as bass
import concourse.mybir as mybir
import concourse.tile as tile
from concourse.bass_utils import run_bass_kernel_spmd

N_CORES = 8
B = 16384
B_LOC = B // N_CORES  # 2048 batch rows per core
NCH = 512  # batch chunk per matmul (fp32 PSUM bank = 512 floats)
NCHUNKS = B_LOC // NCH
KIN = 784  # folded input features (28*28)
NK = 6  # full 128-row k-tiles; tile 6 is the 16-row tail
KTAIL = KIN - NK * 128  # 16
H = 256
NOUT = 10
NWARM = 30  # small PE warm-up matmuls during the DMA fill

F32 = mybir.dt.float32
F16 = mybir.dt.float16
AF = mybir.ActivationFunctionType
ALU = mybir.AluOpType


def build_nc() -> bass.Bass:
    nc = bacc.Bacc(
        "TRN2", target_bir_lowering=False, debug=False, num_devices=N_CORES
    )
    # Host-packed inputs (exact SBUF destination layouts):
    #   xall[ci][p][k*NCH+n] = x_shard[ci*NCH+n, k*128+p]      k = 0..5
    #   xt6[p][c]: cols 0-255 = W1eff[768+p, c]; cols 256+ci*NCH+n =
    #              x_shard[ci*NCH+n, 768+p]                    p < 16
    #   w1[p][k*256+c]  = W1eff[k*128+p, c]   (k < 6; m0|m1 = c 0..255)
    #   wA: cols 0-511 = k-major W2, cols 512-531 = k-major W3
    #   bias cols: 0-1 = b1(m), 2-3 = b2(m), 4 = b3 (first 10 rows)
    xall = nc.dram_tensor("xall", [NCHUNKS, 128, NK * NCH], F16, kind="ExternalInput")
    xt6 = nc.dram_tensor(
        "xt6", [KTAIL, 256 + NCHUNKS * NCH], F16, kind="ExternalInput"
    )
    w1 = nc.dram_tensor("w1", [128, NK * 256], F16, kind="ExternalInput")
    wA = nc.dram_tensor("wA", [128, 2 * H + 2 * NOUT], F16, kind="ExternalInput")
    bias = nc.dram_tensor("bias", [128, 5], F32, kind="ExternalInput")
    outT = nc.dram_tensor("outT", [NOUT, B_LOC], F32, kind="ExternalOutput")

    with tile.TileContext(nc) as tc:
        with (
            tc.tile_pool(name="wgt", bufs=1) as wp,
            tc.tile_pool(name="c0", bufs=1) as cp,
            tc.tile_pool(name="xin", bufs=2) as xp,
            tc.tile_pool(name="act", bufs=3) as hp,
            tc.tile_pool(name="osb", bufs=2) as op,
            tc.tile_pool(name="ps1", bufs=2, space="PSUM") as pp1,
            tc.tile_pool(name="ps2", bufs=2, space="PSUM") as pp2,
        ):
            # PE warm-up: small matmuls on a zeroed scratch tile, no DMA
            # deps: a tiny tile memsets fast (earlier sem -> earlier HAM
            # release) and N=128 gives a fine-grained bridge to data arrival.
            warm = wp.tile([128, 128], F16, name="warm")
            nc.vector.memset(warm[:], 0.0)
            psw = pp1.tile([128, NCH], F32, name="psw", tag="ps1_0")
            for _ in range(NWARM):
                nc.tensor.matmul(
                    psw[:, 0:128], warm[:], warm[:], start=True, stop=True
                )

            # ---- chunk-0-critical loads: 2-3 medium transfers per ring,
            # in exact consumption order ----
            w1A = wp.tile([128, 4 * 256], F16, name="w1A")  # k0..k3 weights
            w1B = wp.tile([128, 2 * 256], F16, name="w1B")  # k4..k5 weights
            t01 = cp.tile([128, 2 * NCH], F16, name="x0_01")
            t23 = cp.tile([128, 2 * NCH], F16, name="x0_23")
            t45 = cp.tile([128, 2 * NCH], F16, name="x0_45")
            t6 = wp.tile([KTAIL, 256 + NCHUNKS * NCH], F16, name="t6")

            # Q1 (sync ring): w1_k0123, x0_k23, w1_k45, then xa prefetches
            nc.sync.dma_start(out=w1A[:], in_=w1[:, 0 : 4 * 256])
            nc.sync.dma_start(out=t23[:], in_=xall[0, :, 2 * NCH : 4 * NCH])
            nc.sync.dma_start(out=w1B[:], in_=w1[:, 4 * 256 : 6 * 256])
            # Q10 (scalar ring): x0_k01, x0_k45, tail, bias, then xb
            nc.scalar.dma_start(out=t01[:], in_=xall[0, :, 0 : 2 * NCH])
            nc.scalar.dma_start(out=t45[:], in_=xall[0, :, 4 * NCH : 6 * NCH])
            was = wp.tile([128, 2 * H + 2 * NOUT], F16, name="was")
            nc.scalar.dma_start(out=t6[:], in_=xt6[:, :])
            bs = wp.tile([128, 5], F32, name="bs")
            nc.scalar.dma_start(out=bs[:], in_=bias[:, :])

            # Per-engine bias staging (consumer then depends on its own
            # engine in program order instead of an extra DMA semaphore).
            ba = wp.tile([128, 5], F32, name="ba")  # ACT's copy
            nc.scalar.activation(ba[:], bs[:], AF.Copy)
            bv = wp.tile([128, 5], F32, name="bv")  # DVE's copy
            nc.vector.tensor_copy(bv[:], bs[:])
            b1a = [ba[:, 0:1], ba[:, 1:2]]
            b2a = [ba[:, 2:3], ba[:, 3:4]]
            b1v = [bv[:, 0:1], bv[:, 1:2]]
            b2v = [bv[:, 2:3], bv[:, 3:4]]
            b3v = bv[0:NOUT, 4:5]

            def x_piece(ci, k, xa_t, xb_t):
                """rhs view for k-tile k of chunk ci."""
                if k == NK:
                    return t6[:, 256 + ci * NCH : 256 + (ci + 1) * NCH]
                if ci == 0:
                    blk, off = (t01, 0) if k < 2 else (t23, 2) if k < 4 else (t45, 4)
                    return blk[:, (k - off) * NCH : (k - off + 1) * NCH]
                if k < 3:
                    return xa_t[:, k * NCH : (k + 1) * NCH]
                return xb_t[:, (k - 3) * NCH : (k - 2) * NCH]

            def w1_piece(k, m):
                if k == NK:
                    return t6[0:KTAIL, m * 128 : (m + 1) * 128]
                if k < 4:
                    return w1A[:, k * 256 + m * 128 : k * 256 + (m + 1) * 128]
                return w1B[:, (k - 4) * 256 + m * 128 : (k - 4) * 256 + (m + 1) * 128]

            # ---- batch-chunk pipeline (x prefetched one chunk ahead) ----
            xa_t = xb_t = None
            for ci in range(NCHUNKS):
                n0 = ci * NCH
                cur = (xa_t, xb_t)

                # layer 1, k-outer/m-inner: each x k-tile is consumed by
                # both m matmuls the moment it lands
                ps1 = [
                    pp1.tile([128, NCH], F32, name="ps1", tag=f"ps1_{m}")
                    for m in range(2)
                ]
                # accumulation order is free: for prefetched chunks, slot
                # the (already-resident) k6 tail before k3 so it covers the
                # xb-arrival wait
                ks = list(range(NK + 1)) if ci == 0 else [0, 1, 2, NK, 3, 4, 5]
                for j, k in enumerate(ks):
                    xv = x_piece(ci, k, *cur)
                    for m in range(2):
                        nc.tensor.matmul(
                            ps1[m][:],
                            w1_piece(k, m),
                            xv,
                            start=(j == 0),
                            stop=(j == NK),
                        )
                if ci + 1 < NCHUNKS:
                    xa_t = xp.tile([128, 3 * NCH], F16, name="xa", tag="xa")
                    nc.sync.dma_start(out=xa_t[:], in_=xall[ci + 1, :, 0 : 3 * NCH])
                    xb_t = xp.tile([128, 3 * NCH], F16, name="xb", tag="xb")
                    nc.scalar.dma_start(
                        out=xb_t[:], in_=xall[ci + 1, :, 3 * NCH : 6 * NCH]
                    )
                if ci == 0:
                    nc.sync.dma_start(out=was[:], in_=wA[:, :])

                h1 = []
                for m in range(2):
                    h = hp.tile([128, NCH], F16, name="h1", tag=f"h1_{m}")
                    if m == 0:
                        nc.scalar.activation(h[:], ps1[m][:], AF.Relu, bias=b1a[m])
                    else:
                        nc.vector.tensor_scalar(
                            h[:], ps1[m][:], b1v[m], 0.0, ALU.add, ALU.max
                        )
                    h1.append(h)

                # layer 2: h2T = relu(W2.T @ h1T + b2)
                h2 = []
                for m in range(2):
                    ps = pp2.tile([128, NCH], F32, name="ps2", tag=f"ps2_{m}")
                    for k in range(2):
                        nc.tensor.matmul(
                            ps[:],
                            was[:, k * H + m * 128 : k * H + (m + 1) * 128],
                            h1[k][:],
                            start=(k == 0),
                            stop=(k == 1),
                        )
                    h = hp.tile([128, NCH], F16, name="h2", tag=f"h2_{m}")
                    if m == 0:
                        nc.scalar.activation(h[:], ps[:], AF.Relu, bias=b2a[m])
                    else:
                        nc.vector.tensor_scalar(
                            h[:], ps[:], b2v[m], 0.0, ALU.add, ALU.max
                        )
                    h2.append(h)

                # layer 3: oT = W3.T @ h2T + b3 (shares ps2_1's bank slots)
                ps3f = pp2.tile([128, NCH], F32, name="ps3", tag="ps2_1")
                ps = ps3f[0:NOUT, :]
                for k in range(2):
                    nc.tensor.matmul(
                        ps,
                        was[:, 2 * H + k * NOUT : 2 * H + (k + 1) * NOUT],
                        h2[k][:],
                        start=(k == 0),
                        stop=(k == 1),
                    )
                ob = op.tile([NOUT, NCH], F32, name="ob", tag="ob")
                nc.vector.tensor_scalar(ob[:], ps, b3v, None, ALU.add)
                # stores ride SWDGE so they never block HWDGE loads; the
                # last one rides the (by-then idle) sync ring
                if ci < NCHUNKS - 1:
                    nc.gpsimd.dma_start(out=outT[:, n0 : n0 + NCH], in_=ob[:])
                else:
                    nc.sync.dma_start(out=outT[:, n0 : n0 + NCH], in_=ob[:])

    nc.compile()
    return nc


def _fold_conv_into_w1(conv_w: np.ndarray, W1: np.ndarray) -> np.ndarray:
    """W1eff[784, 256] such that x @ W1eff == conv_flat(x, conv_w) @ W1."""
    W1v = W1.astype(np.float64).reshape(26, 26, W1.shape[1])
    cw = conv_w.astype(np.float64)
    acc = np.zeros((28, 28, W1.shape[1]), np.float64)
    for di in range(3):
        for dj in range(3):
            acc[di : di + 26, dj : dj + 26, :] += cw[di, dj] * W1v
    return acc.reshape(KIN, W1.shape[1]).astype(np.float32)


def _pack_kmajor(w: np.ndarray, kpad: int) -> np.ndarray:
    """[K, C] -> [128, (K/128)*C] with row-block k at column block k."""
    k, c = w.shape
    wp = np.zeros((kpad, c), w.dtype)
    wp[:k] = w
    return np.ascontiguousarray(
        wp.reshape(kpad // 128, 128, c).transpose(1, 0, 2).reshape(128, -1)
    )


def _run(inputs: dict, trace: bool = False, tmpdir: str | None = None):
    x = np.asarray(inputs["x"], dtype=np.float32)
    w1e = _fold_conv_into_w1(
        np.asarray(inputs["conv_w"]), np.asarray(inputs["W1"])
    ).astype(np.float16)
    w1P = _pack_kmajor(w1e[: NK * 128], NK * 128)  # [128, 6*256]
    w2P = _pack_kmajor(np.asarray(inputs["W2"], np.float16), H)
    w3P = _pack_kmajor(np.asarray(inputs["W3"], np.float16), H)
    wAP = np.ascontiguousarray(np.concatenate([w2P, w3P], axis=1))
    bias = np.zeros((128, 5), np.float32)
    bias[:, 0:2] = np.asarray(inputs["b1"], np.float32).reshape(2, 128).T
    bias[:, 2:4] = np.asarray(inputs["b2"], np.float32).reshape(2, 128).T
    bias[:NOUT, 4] = np.asarray(inputs["b3"], np.float32)

    nc = build_nc()
    in_maps = []
    for c in range(N_CORES):
        xs = x[c * B_LOC : (c + 1) * B_LOC].astype(np.float16)  # [2048, 784]
        # [ci, n, k, p] -> [ci, p, k, n] for the 6 full k-tiles
        arr = xs[:, : NK * 128].reshape(NCHUNKS, NCH, NK, 128)
        xac = np.ascontiguousarray(
            arr.transpose(0, 3, 1, 2).transpose(0, 1, 3, 2).reshape(NCHUNKS, 128, -1)
        )
        xt6c = np.zeros((KTAIL, 256 + NCHUNKS * NCH), np.float16)
        xt6c[:, :256] = w1e[NK * 128 :]
        xt6c[:, 256:] = xs[:, NK * 128 :].T  # [16, 2048]
        in_maps.append(
            {"xall": xac, "xt6": xt6c, "w1": w1P, "wA": wAP, "bias": bias}
        )

    try:
        res = run_bass_kernel_spmd(
            nc, in_maps, list(range(N_CORES)), trace=trace, tmpdir=tmpdir
        )
    except Exception:
        # A prior session can leave a NeuronCore wedged
        # (NRT_EXEC_UNIT_UNRECOVERABLE); a retry with core reset recovers.
        import os

        os.environ.setdefault("NEURON_RT_RESET_CORES", "1")
        res = run_bass_kernel_spmd(
            nc, in_maps, list(range(N_CORES)), trace=trace, tmpdir=tmpdir
        )
    out = np.concatenate([r["outT"].T for r in res.results], axis=0)
    return np.ascontiguousarray(out.astype(np.float32)), res


def kernel(**inputs) -> np.ndarray:
    out, _ = _run(inputs, trace=False)
    return out


# revision 22
# speedup vs baseline: 1.0552x; 1.0020x over previous
"""Trainium2 Bass kernel for nn_DigitConvolutionalModel (3x3 conv + 3-layer MLP).

Math: out = relu(relu(conv3x3(x) @ W1 + b1) @ W2 + b2) @ W3 + b3.

The 3x3 valid conv is linear, so on host we fold it into the first FC:
  h1 = relu(x @ W1eff + b1)  with  W1eff = A @ W1 : [784, 256].
K = 784 is used EXACTLY (6 full 128-row k-tiles + one 16-row tail tile) --
no zero-pad DMA traffic.

Sharding: pure data parallel over the batch across 8 cores (2048 rows each).
Feature-major 3-layer MLP (activations transposed; zero on-device transposes):
  h1T = relu(W1eff.T @ xT + b1)   [256, 2048]
  h2T = relu(W2.T   @ h1T + b2)   [256, 2048]
  oT  =      W3.T   @ h2T + b3    [10, 2048]
Matmuls in fp16 (full-rate PE) with fp32 PSUM accumulation.

DMA discipline (measured): the two HWDGE rings share the same 16 DMA
engines (round-robin per transfer, ~336 GB/s aggregate), each transfer on a
queue costs ~0.5-0.7us of descriptor-generation dead time, and a queue's
first transfer starts ~2.3us after its dma_start issues.  So chunk-0 is
split into just 2-3 medium transfers per ring ordered exactly by first
consumption, the 16-row k6 tail of ALL chunks plus W1's tail rides ONE
small [16, 2304] transfer, and w2|w3 are packed into one tensor.  Nothing
non-critical enters a queue ahead of chunk-0 data.  L1 runs k-outer/m-inner
so both m-tiles consume each arriving x k-tile immediately; warm-up matmuls
on a zeroed tile release the PE HAM clock-gate during the fill.
"""

import numpy as np

import concourse.bacc as bacc
import concourse.bass as bass
import concourse.mybir as mybir
import concourse.tile as tile
from concourse.bass_utils import run_bass_kernel_spmd

N_CORES = 8
B = 16384
B_LOC = B // N_CORES  # 2048 batch rows per core
NCH = 512  # batch chunk per matmul (fp32 PSUM bank = 512 floats)
NCHUNKS = B_LOC // NCH
KIN = 784  # folded input features (28*28)
NK = 6  # full 128-row k-tiles; tile 6 is the 16-row tail
KTAIL = KIN - NK * 128  # 16
H = 256
NOUT = 10
NWARM = 30  # small PE warm-up matmuls during the DMA fill

F32 = mybir.dt.float32
F16 = mybir.dt.float16
AF = mybir.ActivationFunctionType
ALU = mybir.AluOpType


def build_nc() -> bass.Bass:
    nc = bacc.Bacc(
        "TRN2", target_bir_lowering=False, debug=False, num_devices=N_CORES
    )
    # Host-packed inputs (exact SBUF destination layouts):
    #   xall[ci][p][k*NCH+n] = x_shard[ci*NCH+n, k*128+p]      k = 0..5
    #   xt6[p][c]: cols 0-255 = W1eff[768+p, c]; cols 256+ci*NCH+n =
    #              x_shard[ci*NCH+n, 768+p]                    p < 16
    #   w1[p][k*256+c]  = W1eff[k*128+p, c]   (k < 6; m0|m1 = c 0..255)
    #   wA: cols 0-511 = k-major W2, cols 512-531 = k-major W3
    #   bias cols: 0-1 = b1(m), 2-3 = b2(m), 4 = b3 (first 10 rows)
    xall = nc.dram_tensor("xall", [NCHUNKS, 128, NK * NCH], F16, kind="ExternalInput")
    xt6 = nc.dram_tensor(
        "xt6", [KTAIL, 256 + NCHUNKS * NCH], F16, kind="ExternalInput"
    )
    w1 = nc.dram_tensor("w1", [128, NK * 256], F16, kind="ExternalInput")
    wA = nc.dram_tensor("wA", [128, 2 * H + 2 * NOUT], F16, kind="ExternalInput")
    bias = nc.dram_tensor("bias", [128, 5], F32, kind="ExternalInput")
    outT = nc.dram_tensor("outT", [NOUT, B_LOC], F32, kind="ExternalOutput")

    with tile.TileContext(nc) as tc:
        with (
            tc.tile_pool(name="wgt", bufs=1) as wp,
            tc.tile_pool(name="c0", bufs=1) as cp,
            tc.tile_pool(name="xin", bufs=2) as xp,
            tc.tile_pool(name="act", bufs=3) as hp,
            tc.tile_pool(name="osb", bufs=2) as op,
            tc.tile_pool(name="ps1", bufs=2, space="PSUM") as pp1,
            tc.tile_pool(name="ps2", bufs=2, space="PSUM") as pp2,
        ):
            # PE warm-up: small matmuls on a zeroed scratch tile, no DMA
            # deps: a tiny tile memsets fast (earlier sem -> earlier HAM
            # release) and N=128 gives a fine-grained bridge to data arrival.
            warm = wp.tile([128, 128], F16, name="warm")
            nc.vector.memset(warm[:], 0.0)
            psw = pp1.tile([128, NCH], F32, name="psw", tag="ps1_0")
            for _ in range(NWARM):
                nc.tensor.matmul(
                    psw[:, 0:128], warm[:], warm[:], start=True, stop=True
                )

            # ---- chunk-0-critical loads: 2-3 medium transfers per ring,
            # in exact consumption order ----
            w1A = wp.tile([128, 4 * 256], F16, name="w1A")  # k0..k3 weights
            w1B = wp.tile([128, 2 * 256], F16, name="w1B")  # k4..k5 weights
            t01 = cp.tile([128, 2 * NCH], F16, name="x0_01")
            t23 = cp.tile([128, 2 * NCH], F16, name="x0_23")
            t45 = cp.tile([128, 2 * NCH], F16, name="x0_45")
            t6 = wp.tile([KTAIL, 256 + NCHUNKS * NCH], F16, name="t6")

            # Q1 (sync ring): w1_k0123, x0_k23, w1_k45, then xa prefetches
            nc.sync.dma_start(out=w1A[:], in_=w1[:, 0 : 4 * 256])
            nc.sync.dma_start(out=t23[:], in_=xall[0, :, 2 * NCH : 4 * NCH])
            nc.sync.dma_start(out=w1B[:], in_=w1[:, 4 * 256 : 6 * 256])
            # Q10 (scalar ring): x0_k01, x0_k45, tail, bias, then xb
            nc.scalar.dma_start(out=t01[:], in_=xall[0, :, 0 : 2 * NCH])
            nc.scalar.dma_start(out=t45[:], in_=xall[0, :, 4 * NCH : 6 * NCH])
            was = wp.tile([128, 2 * H + 2 * NOUT], F16, name="was")
            nc.scalar.dma_start(out=t6[:], in_=xt6[:, :])
            bs = wp.tile([128, 5], F32, name="bs")
            nc.scalar.dma_start(out=bs[:], in_=bias[:, :])

            # Per-engine bias staging (consumer then depends on its own
            # engine in program order instead of an extra DMA semaphore).
            ba = wp.tile([128, 5], F32, name="ba")  # ACT's copy
            nc.scalar.activation(ba[:], bs[:], AF.Copy)
            bv = wp.tile([128, 5], F32, name="bv")  # DVE's copy
            nc.vector.tensor_copy(bv[:], bs[:])
            b1a = [ba[:, 0:1], ba[:, 1:2]]
            b2a = [ba[:, 2:3], ba[:, 3:4]]
            b1v = [bv[:, 0:1], bv[:, 1:2]]
            b2v = [bv[:, 2:3], bv[:, 3:4]]
            b3v = bv[0:NOUT, 4:5]

            def x_piece(ci, k, xa_t, xb_t):
                """rhs view for k-tile k of chunk ci."""
                if k == NK:
                    return t6[:, 256 + ci * NCH : 256 + (ci + 1) * NCH]
                if ci == 0:
                    blk, off = (t01, 0) if k < 2 else (t23, 2) if k < 4 else (t45, 4)
                    return blk[:, (k - off) * NCH : (k - off + 1) * NCH]
                if k < 3:
                    return xa_t[:, k * NCH : (k + 1) * NCH]
                return xb_t[:, (k - 3) * NCH : (k - 2) * NCH]

            def w1_piece(k, m):
                if k == NK:
                    return t6[0:KTAIL, m * 128 : (m + 1) * 128]
                if k < 4:
                    return w1A[:, k * 256 + m * 128 : k * 256 + (m + 1) * 128]
                return w1B[:, (k - 4) * 256 + m * 128 : (k - 4) * 256 + (m + 1) * 128]

            # ---- batch-chunk pipeline (x prefetched one chunk ahead) ----
            xa_t = xb_t = None
            for ci in range(NCHUNKS):
                n0 = ci * NCH
                cur = (xa_t, xb_t)

                # layer 1, k-outer/m-inner: each x k-tile is consumed by
                # both m matmuls the moment it lands
                ps1 = [
                    pp1.tile([128, NCH], F32, name="ps1", tag=f"ps1_{m}")
                    for m in range(2)
                ]
                # accumulation order is free: for prefetched chunks, slot
                # the (already-resident) k6 tail before k3 so it covers the
                # xb-arrival wait.  The LAST chunk runs m-outer instead so
                # ps1[0] completes 7 matmuls early -- its relu overlaps the
                # m1 pass and L2 starts right at L1-end (shorter tail).
                if ci < NCHUNKS - 1:
                    ks = list(range(NK + 1)) if ci == 0 else [0, 1, 2, NK, 3, 4, 5]
                    for j, k in enumerate(ks):
                        xv = x_piece(ci, k, *cur)
                        for m in range(2):
                            nc.tensor.matmul(
                                ps1[m][:],
                                w1_piece(k, m),
                                xv,
                                start=(j == 0),
                                stop=(j == NK),
                            )
                else:
                    for m in range(2):
                        for j in range(NK + 1):
                            nc.tensor.matmul(
                                ps1[m][:],
                                w1_piece(j, m),
                                x_piece(ci, j, *cur),
                                start=(j == 0),
                                stop=(j == NK),
                            )
                if ci + 1 < NCHUNKS:
                    xa_t = xp.tile([128, 3 * NCH], F16, name="xa", tag="xa")
                    nc.sync.dma_start(out=xa_t[:], in_=xall[ci + 1, :, 0 : 3 * NCH])
                    xb_t = xp.tile([128, 3 * NCH], F16, name="xb", tag="xb")
                    nc.scalar.dma_start(
                        out=xb_t[:], in_=xall[ci + 1, :, 3 * NCH : 6 * NCH]
                    )
                if ci == 0:
                    nc.sync.dma_start(out=was[:], in_=wA[:, :])

                h1 = []
                for m in range(2):
                    h = hp.tile([128, NCH], F16, name="h1", tag=f"h1_{m}")
                    if m == 0:
                        nc.scalar.activation(h[:], ps1[m][:], AF.Relu, bias=b1a[m])
                    else:
                        nc.vector.tensor_scalar(
                            h[:], ps1[m][:], b1v[m], 0.0, ALU.add, ALU.max
                        )
                    h1.append(h)

                # layer 2: h2T = relu(W2.T @ h1T + b2)
                h2 = []
                for m in range(2):
                    ps = pp2.tile([128, NCH], F32, name="ps2", tag=f"ps2_{m}")
                    for k in range(2):
                        nc.tensor.matmul(
                            ps[:],
                            was[:, k * H + m * 128 : k * H + (m + 1) * 128],
                            h1[k][:],
                            start=(k == 0),
                            stop=(k == 1),
                        )
                    h = hp.tile([128, NCH], F16, name="h2", tag=f"h2_{m}")
                    if m == 0:
                        nc.scalar.activation(h[:], ps[:], AF.Relu, bias=b2a[m])
                    else:
                        nc.vector.tensor_scalar(
                            h[:], ps[:], b2v[m], 0.0, ALU.add, ALU.max
                        )
                    h2.append(h)

                # layer 3: oT = W3.T @ h2T + b3 (shares ps2_1's bank slots)
                ps3f = pp2.tile([128, NCH], F32, name="ps3", tag="ps2_1")
                ps = ps3f[0:NOUT, :]
                for k in range(2):
                    nc.tensor.matmul(
                        ps,
                        was[:, 2 * H + k * NOUT : 2 * H + (k + 1) * NOUT],
                        h2[k][:],
                        start=(k == 0),
                        stop=(k == 1),
                    )
                ob = op.tile([NOUT, NCH], F32, name="ob", tag="ob")
                nc.vector.tensor_scalar(ob[:], ps, b3v, None, ALU.add)
                # stores ride SWDGE so they never block HWDGE loads; the
                # last one rides the (by-then idle) sync ring
                if ci < NCHUNKS - 1:
                    nc.gpsimd.dma_start(out=outT[:, n0 : n0 + NCH], in_=ob[:])
                else:
                    nc.sync.dma_start(out=outT[:, n0 : n0 + NCH], in_=ob[:])

    nc.compile()
    return nc


def _fold_conv_into_w1(conv_w: np.ndarray, W1: np.ndarray) -> np.ndarray:
    """W1eff[784, 256] such that x @ W1eff == conv_flat(x, conv_w) @ W1."""
    W1v = W1.astype(np.float64).reshape(26, 26, W1.shape[1])
    cw = conv_w.astype(np.float64)
    acc = np.zeros((28, 28, W1.shape[1]), np.float64)
    for di in range(3):
        for dj in range(3):
            acc[di : di + 26, dj : dj + 26, :] += cw[di, dj] * W1v
    return acc.reshape(KIN, W1.shape[1]).astype(np.float32)


def _pack_kmajor(w: np.ndarray, kpad: int) -> np.ndarray:
    """[K, C] -> [128, (K/128)*C] with row-block k at column block k."""
    k, c = w.shape
    wp = np.zeros((kpad, c), w.dtype)
    wp[:k] = w
    return np.ascontiguousarray(
        wp.reshape(kpad // 128, 128, c).transpose(1, 0, 2).reshape(128, -1)
    )


def _run(inputs: dict, trace: bool = False, tmpdir: str | None = None):
    x = np.asarray(inputs["x"], dtype=np.float32)
    w1e = _fold_conv_into_w1(
        np.asarray(inputs["conv_w"]), np.asarray(inputs["W1"])
    ).astype(np.float16)
    w1P = _pack_kmajor(w1e[: NK * 128], NK * 128)  # [128, 6*256]
    w2P = _pack_kmajor(np.asarray(inputs["W2"], np.float16), H)
    w3P = _pack_kmajor(np.asarray(inputs["W3"], np.float16), H)
    wAP = np.ascontiguousarray(np.concatenate([w2P, w3P], axis=1))
    bias = np.zeros((128, 5), np.float32)
    bias[:, 0:2] = np.asarray(inputs["b1"], np.float32).reshape(2, 128).T
    bias[:, 2:4] = np.asarray(inputs["b2"], np.float32).reshape(2, 128).T
    bias[:NOUT, 4] = np.asarray(inputs["b3"], np.float32)

    nc = build_nc()
    in_maps = []
    for c in range(N_CORES):
        xs = x[c * B_LOC : (c + 1) * B_LOC].astype(np.float16)  # [2048, 784]
        # [ci, n, k, p] -> [ci, p, k, n] for the 6 full k-tiles
        arr = xs[:, : NK * 128].reshape(NCHUNKS, NCH, NK, 128)
        xac = np.ascontiguousarray(
            arr.transpose(0, 3, 1, 2).transpose(0, 1, 3, 2).reshape(NCHUNKS, 128, -1)
        )
        xt6c = np.zeros((KTAIL, 256 + NCHUNKS * NCH), np.float16)
        xt6c[:, :256] = w1e[NK * 128 :]
        xt6c[:, 256:] = xs[:, NK * 128 :].T  # [16, 2048]
        in_maps.append(
            {"xall": xac, "xt6": xt6c, "w1": w1P, "wA": wAP, "bias": bias}
        )

    try:
        res = run_bass_kernel_spmd(
            nc, in_maps, list(range(N_CORES)), trace=trace, tmpdir=tmpdir
        )
    except Exception:
        # A prior session can leave a NeuronCore wedged
        # (NRT_EXEC_UNIT_UNRECOVERABLE); a retry with core reset recovers.
        import os

        os.environ.setdefault("NEURON_RT_RESET_CORES", "1")
        res = run_bass_kernel_spmd(
            nc, in_maps, list(range(N_CORES)), trace=trace, tmpdir=tmpdir
        )
    out = np.concatenate([r["outT"].T for r in res.results], axis=0)
    return np.ascontiguousarray(out.astype(np.float32)), res


def kernel(**inputs) -> np.ndarray:
    out, _ = _run(inputs, trace=False)
    return out
